# revision 7
# baseline (speedup 1.0000x reference)
"""Trainium2 Bass kernel for nn_DTASNNEmbedding (gated spiking conv recurrence).

Strategy: data-parallel over batch B=8 across 8 NeuronCores (one element per
core). Per core and time step, the two 3x3 convs (events->2C and spike->2C)
are merged into one 128->128-channel conv: the contraction packs
[spike(64ch); events(64ch)] on SBUF partitions, and each of the 9 taps is a
[128,128]x[128,N] matmul accumulating into PSUM. Weights are split
w = hi + lo (bf16 pair) so two 1-cycle/row passes give fp32-class accuracy
(activations are exact binary in bf16). Images live in a flat padded layout
(row stride 65, zero guard columns) so tap shifts are pure AP offsets.
The gated-LIF pointwise runs on ScalarE (sigmoid), VectorE (mult/add/
threshold/reset) and GpSimd (spike-count accumulation), overlapped with the
TensorE conv of neighboring chunks.
"""

from contextlib import ExitStack

import numpy as np
import ml_dtypes

import concourse.bass as bass
import concourse.tile as tile
from concourse import mybir
from concourse.bass_utils import run_bass_kernel_spmd

T = 5
C = 64            # Cout = Cin = 64
HW = 64           # H = W
RS = HW + 1       # row stride (64 px + shared zero pad col)
SPAN = HW * RS    # 4160
G = RS + 1        # 66-col zero guard front/back
L = G + SPAN + G  # combo buffer length 4292
NCHUNK = 10
CW = SPAN // NCHUNK  # 416 psum columns per chunk
TAPS = [(dy, dx) for dy in (-1, 0, 1) for dx in (-1, 0, 1)]
N_CORES = 8

F32 = mybir.dt.float32
DT_RHS = mybir.dt.bfloat16
BF16 = np.dtype(ml_dtypes.bfloat16)


def _build_program(with_bias: bool):
    nc = bass.Bass("TRN2", target_bir_lowering=False, debug=False)

    ev_d = nc.dram_tensor("ev", [T, C, HW, HW], DT_RHS, kind="ExternalInput").ap()
    wall_d = nc.dram_tensor("w_all", [128, 18 * 128], DT_RHS, kind="ExternalInput").ap()
    th_d = nc.dram_tensor("th", [C, 8], F32, kind="ExternalInput").ap()
    if with_bias:
        wb_d = nc.dram_tensor("wb", [2, 128], DT_RHS, kind="ExternalInput").ap()
    out_d = nc.dram_tensor("out", [C, HW, HW], F32, kind="ExternalOutput").ap()

    with tile.TileContext(nc) as tc, ExitStack() as ctx:
        state = ctx.enter_context(tc.tile_pool(name="state", bufs=1))
        gpool = ctx.enter_context(tc.tile_pool(name="gt", bufs=3))
        pspool = ctx.enter_context(tc.tile_pool(name="ps", bufs=4, space="PSUM"))

        w_all = state.tile([128, 18 * 128], DT_RHS, name="w_all_sb")
        w_hi = w_all[:, 0 : 9 * 128]
        w_lo = w_all[:, 9 * 128 : 18 * 128]
        th_sb = state.tile([C, 8], F32, name="th_sb")
        combo = [
            state.tile([128, L], DT_RHS, name="combo0"),
            state.tile([128, L], DT_RHS, name="combo1"),
        ]
        vmem = state.tile([C, SPAN], F32, name="vmem")
        ssum = state.tile([C, SPAN], F32, name="ssum")
        zero64 = state.tile([C, CW], F32, name="zero64")
        if with_bias:
            wb_sb = state.tile([2, 128], DT_RHS, name="wb_sb")
            ones_sb = state.tile([2, CW], DT_RHS, name="ones_sb")

        nc.sync.dma_start(w_all[:, :], wall_d[:, :])
        nc.sync.dma_start(th_sb[:, :], th_d[:, :])
        if with_bias:
            nc.sync.dma_start(wb_sb[:, :], wb_d[:, :])
            nc.gpsimd.memset(ones_sb[:, :], 1.0)
        nc.vector.memset(combo[0][:, :], 0.0)
        nc.vector.memset(combo[1][:, :], 0.0)
        nc.vector.memset(vmem[:, :], 0.0)
        nc.vector.memset(ssum[:, :], 0.0)
        nc.vector.memset(zero64[:, :], 0.0)

        def x_view(buf):
            return buf[64:128, G : G + SPAN].rearrange(
                "p (r w) -> p r w", w=RS
            )[:, :, 0:HW]

        nc.sync.dma_start(x_view(combo[0]), ev_d[0, :, :, :])

        for t in range(T):
            src = combo[t % 2]
            dst = combo[(t + 1) % 2]
            if t + 1 < T:
                nc.sync.dma_start(x_view(dst), ev_d[t + 1, :, :, :])

            for c in range(NCHUNK):
                p0 = G + CW * c
                ps = pspool.tile([128, CW], F32, name="ps")
                n_mm = 18 + (1 if with_bias else 0)
                i_mm = 0
                for half in (0, 1):
                    for k, (dy, dx) in enumerate(TAPS):
                        d = dy * RS + dx
                        kk = half * 9 + k
                        nc.tensor.matmul(
                            ps[:, :],
                            lhsT=w_all[:, kk * 128 : (kk + 1) * 128],
                            rhs=src[:, p0 + d : p0 + d + CW],
                            start=(i_mm == 0),
                            stop=(i_mm == n_mm - 1),
                        )
                        i_mm += 1
                if with_bias:
                    nc.tensor.matmul(
                        ps[:, :], lhsT=wb_sb[0:2, :], rhs=ones_sb[0:2, :],
                        start=False, stop=True,
                    )

                w0, w1 = CW * c, CW * (c + 1)
                gt = gpool.tile([128, CW], F32, name="gt")
                # Evict gate-pre to SBUF on DVE (so DVE is the only PSUM
                # reader -> fewer sem waits on the matmuls), DMA-move it to
                # partitions 0-63 (walrus requires compute operands on the
                # same partitions; DMA is the legal partition mover), then
                # sigmoid in place.
                nc.vector.tensor_copy(gt[64:128, :], ps[64:128, :])
                nc.sync.dma_start(gt[0:64, :], gt[64:128, :])
                nc.scalar.activation(
                    gt[0:64, :], gt[0:64, :], mybir.ActivationFunctionType.Sigmoid
                )
                nc.vector.tensor_tensor(
                    gt[0:64, :], gt[0:64, :], vmem[:, w0:w1], mybir.AluOpType.mult
                )
                nc.vector.tensor_tensor(
                    vmem[:, w0:w1], gt[0:64, :], ps[0:64, :], mybir.AluOpType.add
                )
                nc.vector.tensor_scalar(
                    dst[0:64, p0 : p0 + CW], vmem[:, w0:w1],
                    th_sb[:, t : t + 1], None, mybir.AluOpType.is_ge,
                )
                nc.vector.copy_predicated(
                    vmem[:, w0:w1],
                    dst[0:64, p0 : p0 + CW].bitcast(mybir.dt.uint16),
                    zero64[:, :],
                )
                nc.gpsimd.tensor_tensor(
                    ssum[:, w0:w1], ssum[:, w0:w1], dst[0:64, p0 : p0 + CW],
                    mybir.AluOpType.add,
                )
                pads = [r for r in range(HW)
                        if CW * c <= r * RS + HW < CW * (c + 1)]
                if pads:
                    r0, n = pads[0], len(pads)
                    start = G + r0 * RS + HW
                    pad_ap = dst[0:64, start : start + n * RS].rearrange(
                        "p (r w) -> p r w", w=RS
                    )[:, :, 0:1]
                    nc.vector.memset(pad_ap, 0.0)

        ov = ssum[:, :].rearrange("p (r w) -> p r w", w=RS)[:, :, 0:HW]
        nc.sync.dma_start(out_d[:, 0:32, :], ov[:, 0:32, :])
        nc.sync.dma_start(out_d[:, 32:64, :], ov[:, 32:64, :])

    _split_sync_waits(nc, cap=1)
    return nc


_ENGINE_ATTR = {
    "EngineType.PE": "tensor",
    "EngineType.DVE": "vector",
    "EngineType.Activation": "scalar",
    "EngineType.Pool": "gpsimd",
    "EngineType.SP": "sync",
    "EngineType.SyncIO": "sync",
}


def _split_sync_waits(nc, cap=1):
    """The walrus build here accepts only `cap` semaphore-wait commands per
    instruction ("Too many sync wait commands" otherwise). Hoist surplus
    waits onto same-engine NOPs inserted immediately before the carrier --
    identical wait-position semantics, so no scheduling change."""
    fn = nc.m.functions[0]
    blocks = list(fn.blocks)
    lists = [bb.instructions for bb in blocks]
    work = []  # (bb_list_index, position, ins)
    for bi, il in enumerate(lists):
        for pos, ins in enumerate(il):
            si = ins.sync_info
            if si is not None and si.on_wait and len(si.on_wait) > cap:
                work.append((bi, pos, ins))
    if not work:
        return
    created = []
    for _, _, ins in work:
        ws = ins.sync_info.on_wait
        n_extra = (len(ws) + cap - 1) // cap - 1
        eng = _ENGINE_ATTR[str(ins.engine)]
        created.append(
            [getattr(nc, eng).nop(hint="wsplit").ins for _ in range(n_extra)]
        )
    created_ids = {id(n) for ns in created for n in ns}
    # remove the freshly appended nops from wherever bass put them
    for bb in list(fn.blocks):
        il = bb.instructions
        if any(id(i) in created_ids for i in il):
            il[:] = [i for i in il if id(i) not in created_ids]
    # insert per-bb in reverse position order so indices stay valid
    from concourse import mybir as _mb

    for (bi, pos, ins), nops in sorted(
        zip(work, created), key=lambda x: (x[0][0], -x[0][1])
    ):
        il = lists[bi]
        assert il[pos] is ins, "instruction moved during wait splitting"
        ws = list(ins.sync_info.on_wait)
        chunks = [ws[k : k + cap] for k in range(0, len(ws), cap)]
        ins.sync_info.on_wait[:] = chunks[-1]
        for nop, ch in zip(nops, chunks[:-1]):
            nop.sync_info = _mb.SyncInfo(on_wait=list(ch), on_update=[])
        il[pos:pos] = nops


_prog_cache: dict[bool, bass.Bass] = {}


def _get_program(with_bias: bool):
    if with_bias not in _prog_cache:
        _prog_cache[with_bias] = _build_program(with_bias)
    return _prog_cache[with_bias]


def _prep_shared(w_in, b_in, w_gate, b_gate, thresh_decay, with_bias):
    # lhsT tables: combo partitions 0-63 = spike (w_gate), 64-127 = events
    # (w_in); lhsT columns = out channels [cur(64) | gate-pre(64)].
    w_cat = np.concatenate(
        [np.asarray(w_gate, np.float64), np.asarray(w_in, np.float64)], axis=1
    )  # [128co, 128ci, 3, 3]
    hi_l, lo_l = [], []
    for (dy, dx) in TAPS:
        w_tap = w_cat[:, :, dy + 1, dx + 1]  # [co, ci]
        hi = w_tap.astype(BF16)
        lo = (w_tap - hi.astype(np.float64)).astype(BF16)
        hi_l.append(hi.T)
        lo_l.append(lo.T)
    w_all = np.ascontiguousarray(np.concatenate(hi_l + lo_l, axis=1))

    th = np.zeros((C, 8), np.float32)
    td = np.asarray(thresh_decay, np.float64)
    for t in range(T):
        th[:, t] = (1.0 * td**t).astype(np.float32)

    shared = {"w_all": w_all, "th": th}
    if with_bias:
        b = np.asarray(b_in, np.float64) + np.asarray(b_gate, np.float64)
        wb = np.zeros((2, 128), np.float64)
        wb[0] = b.astype(BF16).astype(np.float64)
        wb[1] = (b - wb[0]).astype(BF16).astype(np.float64)
        shared["wb"] = wb.astype(BF16)
    return shared


def _run(events, w_in, b_in, w_gate, b_gate, thresh_decay, trace=False, **trace_kw):
    events = np.asarray(events)
    B = events.shape[0]
    assert B == N_CORES and events.shape[1] == T

    with_bias = bool(np.any(np.asarray(b_in)) or np.any(np.asarray(b_gate)))
    nc = _get_program(with_bias)
    shared = _prep_shared(w_in, b_in, w_gate, b_gate, thresh_decay, with_bias)

    in_maps = []
    for b in range(B):
        ev = np.ascontiguousarray(events[b, ::-1]).astype(BF16)  # time-reversed
        m = dict(shared)
        m["ev"] = ev
        in_maps.append(m)

    res = run_bass_kernel_spmd(
        nc, in_maps, core_ids=list(range(N_CORES)), trace=trace, **trace_kw
    )
    out = np.stack(
        [np.asarray(res.results[b]["out"], np.float64) for b in range(B)]
    )
    out = (out / T).astype(np.float32)
    return out, res


def kernel(events, w_in, b_in, w_gate, b_gate, thresh_decay):
    out, _ = _run(events, w_in, b_in, w_gate, b_gate, thresh_decay, trace=False)
    return out
